# revision 9
# baseline (speedup 1.0000x reference)
"""Trainium2 Bass kernel for GQA attention (32 q heads / 16 kv heads, head_dim
128, L=2048, D=4608) with RoPE, tanh softcap 50, causal mask, o_proj.

Tensor-parallel over heads across 8 NeuronCores; core c owns q-heads 4c..4c+3
and kv-heads 2c..2c+1; host sums the 8 partial [L, D] outputs.

v2: single fused software-pipelined schedule ("weave").  The kernel runs 4
rounds (one per 512-wide q chunk).  In round r the PE instruction stream
interleaves three workloads so the scalar-engine softcap chain (tanh+exp,
~178us total) always hides under ready matmul work:
  B: attention for q-chunk r (scores -> tanh -> exp -> mask -> PV)
  A: Q/K/V projections + rope for q-chunk r+1 (16 chains of 36 MMs @256)
  C: o_proj for q-chunk r-1 (36 groups of 4 MMs @512)

Other changes vs v1:
  - PV computes attnT [d, q] directly (lhsT = V tile, rhs = PT tile, 512-wide
    accumulation in one PSUM bank) - no PE transposes, no 129-wide matmuls.
  - softmax denominator: DVE accumulates sum of exp tiles in f32, one
    ones-vector matmul per (head, chunk) reduces over partitions; the
    reciprocal is broadcast (gpsimd) and folded into the attnT PSUM drain.
  - rope applied by DVE reading the projection PSUM directly (no ACT copies).
  - x streamed in 256-wide column blocks (halves SBUF double-buffer cost).
"""

import numpy as np
import ml_dtypes

import concourse.bass as bass
import concourse.mybir as mybir
import concourse.tile as tile
from concourse import bacc

F32 = mybir.dt.float32
BF16 = mybir.dt.bfloat16
BF16_NP = ml_dtypes.bfloat16
AF = mybir.ActivationFunctionType

N_HEADS = 32
N_KV = 16
HEAD_DIM = 128
ROPE_THETA = 10000.0
SOFTCAP = 50.0
SCALE = 1.0 / 12.0  # 1/sqrt(144)
L = 2048
D = 4608
N_CORES = 8
QH = N_HEADS // N_CORES        # 4 local q heads
KVH = N_KV // N_CORES          # 2 local kv heads
KC = D // 128                  # 36 contraction chunks
NB = L // 256                  # 8 projection column blocks (256 wide)
NQ = L // 512                  # 4 attention q-chunks (512 wide)
LT = L // 128                  # 16 k-tiles of 128
DOUT_CHUNKS = D // 512         # 9 o_proj output chunks
ATT_LAG = 2                    # attnT MM trails the exp by this many tiles


def _emit(nc):
    xt_d = nc.dram_tensor("xt", [D, L], BF16, kind="ExternalInput")
    wqt_d = nc.dram_tensor("wqt", [D, QH * 128], BF16, kind="ExternalInput")
    wkt_d = nc.dram_tensor("wkt", [D, KVH * 128], BF16, kind="ExternalInput")
    wvt_d = nc.dram_tensor("wvt", [D, KVH * 128], BF16, kind="ExternalInput")
    wot_d = nc.dram_tensor("wot", [QH * 128, D], BF16, kind="ExternalInput")
    cost_d = nc.dram_tensor("cost", [128, L], BF16, kind="ExternalInput")
    sint_d = nc.dram_tensor("sint", [128, L], BF16, kind="ExternalInput")
    mask_d = nc.dram_tensor("mask", [128, 512], BF16, kind="ExternalInput")
    out_d = nc.dram_tensor("out", [L, D], F32, kind="ExternalOutput")

    from contextlib import ExitStack

    with tile.TileContext(nc) as tc:
        with ExitStack() as stack:
            pool = lambda *a, **kw: stack.enter_context(tc.tile_pool(*a, **kw))
            const = pool(name="const", bufs=1)
            persist = pool(name="persist", bufs=1)
            qtn = pool(name="qtn", bufs=2)
            xcol = pool(name="xcol", bufs=2)
            wts = pool(name="wts", bufs=1)
            wop = pool(name="wop", bufs=1)
            rtmp = pool(name="rtmp", bufs=1)
            ttp = pool(name="ttp", bufs=2)
            ptp = pool(name="ptp", bufs=3)
            accp = pool(name="accp", bufs=2)
            atsb = pool(name="atsb", bufs=2)
            rsb = pool(name="rsb", bufs=1)
            rbc = pool(name="rbc", bufs=1)
            ost = pool(name="ost", bufs=2)
            pj_ps = pool(name="pj_ps", bufs=2, space="PSUM")
            sc_ps = pool(name="sc_ps", bufs=2, space="PSUM")
            at_ps = pool(name="at_ps", bufs=2, space="PSUM")
            op_ps = pool(name="op_ps", bufs=2, space="PSUM")
            # ---------------- constants ----------------
            ones = const.tile([128, 1], F32)
            nc.vector.memset(ones[:], 1.0)
            warm = const.tile([128, 2], F32)
            # preload the exp/tanh activation table set during the prologue
            nc.scalar.activation(warm[:, 0:1], ones[:], AF.Tanh)
            nc.scalar.activation(warm[:, 1:2], warm[:, 0:1], AF.Exp)

            cost = const.tile([128, L], BF16)
            sint = const.tile([128, L], BF16)
            mask0 = const.tile([128, 512], BF16)

            # persistent per-head tensors
            KT = [persist.tile([128, L], BF16, tag=f"kt{g}", name=f"kt{g}")
                  for g in range(KVH)]
            V = [persist.tile([128, LT * 128], BF16, tag=f"v{g}", name=f"v{g}")
                 for g in range(KVH)]

            # ---------------- input DMAs (prologue) ----------------
            wk, wq, wv = [], [], []
            xc_store = {}  # block -> [36 chunk APs]

            def dma_x_block(b):
                aps = []
                for k in range(KC):
                    t = xcol.tile([128, 256], BF16, tag=f"x{k}", name=f"xc{k}")
                    nc.sync.dma_start(
                        t[:], xt_d[k * 128:(k + 1) * 128, b * 256:(b + 1) * 256])
                    aps.append(t)
                xc_store[b] = aps

            # first K chain needs wk[k] + block0; interleave those DMAs
            for k in range(KC):
                w = wts.tile([128, KVH * 128], BF16, tag=f"k{k}", name=f"wk{k}")
                nc.sync.dma_start(w[:], wkt_d[k * 128:(k + 1) * 128, :])
                wk.append(w)
            dma_x_block(0)
            nc.sync.dma_start(cost[:], cost_d[:])
            nc.sync.dma_start(sint[:], sint_d[:])
            for k in range(KC):
                w = wts.tile([128, QH * 128], BF16, tag=f"q{k}", name=f"wq{k}")
                nc.sync.dma_start(w[:], wqt_d[k * 128:(k + 1) * 128, :])
                wq.append(w)
            dma_x_block(1)
            for k in range(KC):
                w = wts.tile([128, KVH * 128], BF16, tag=f"v{k}", name=f"wv{k}")
                nc.sync.dma_start(w[:], wvt_d[k * 128:(k + 1) * 128, :])
                wv.append(w)
            nc.sync.dma_start(mask0[:], mask_d[:])
            WO = []
            for h in range(QH):
                w = wop.tile([128, D], BF16, tag=f"wo{h}")
                nc.sync.dma_start(w[:], wot_d[h * 128:(h + 1) * 128, :])
                WO.append(w)

            # ---------------- projection chains ----------------
            qtn_cur = {}   # h -> current-round QT tile [128, 512]

            def rope_drain(ps, dst, b):
                cols = slice(b * 256, (b + 1) * 256)
                m1 = rtmp.tile([128, 256], F32, tag="m1", name="m1")
                nc.vector.tensor_mul(m1[0:64, :], ps[64:128, 0:256], sint[0:64, cols])
                nc.vector.tensor_mul(m1[64:128, :], ps[0:64, 0:256], sint[64:128, cols])
                m2 = rtmp.tile([128, 256], F32, tag="m2", name="m2")
                nc.vector.tensor_mul(m2[:], ps[:, 0:256], cost[:, cols])
                nc.vector.tensor_add(dst, m2[:], m1[:])

            def k_chain(g, b):
                xc = xc_store[b]
                ps = pj_ps.tile([128, 512], F32, tag="pj", name="ps")
                for k in range(KC):
                    nc.tensor.matmul(
                        ps[:, 0:256], wk[k][:, g * 128:(g + 1) * 128], xc[k][:],
                        start=(k == 0), stop=(k == KC - 1))
                rope_drain(ps, KT[g][:, b * 256:(b + 1) * 256], b)

            def q_chain(h, b):
                xc = xc_store[b]
                if b % 2 == 0:
                    qtn_cur[h] = qtn.tile([128, 512], BF16, tag=f"q{h}", name=f"qt{h}")
                ps = pj_ps.tile([128, 512], F32, tag="pj", name="ps")
                for k in range(KC):
                    nc.tensor.matmul(
                        ps[:, 0:256], wq[k][:, h * 128:(h + 1) * 128], xc[k][:],
                        start=(k == 0), stop=(k == KC - 1))
                c0 = (b % 2) * 256
                rope_drain(ps, qtn_cur[h][:, c0:c0 + 256], b)

            def v_chain(sub, b):
                xc = xc_store[b]
                ps = pj_ps.tile([128, 512], F32, tag="pj", name="ps")
                for k in range(KC):
                    nc.tensor.matmul(
                        ps[:, 0:256], xc[k][:, sub * 128:(sub + 1) * 128], wv[k][:],
                        start=(k == 0), stop=(k == KC - 1))
                mk = 2 * b + sub
                for g in range(KVH):
                    nc.vector.tensor_copy(
                        V[g][:, mk * 128:(mk + 1) * 128],
                        ps[:, g * 128:(g + 1) * 128])

            def block_chains(b):
                ch = []
                for g in range(KVH):
                    ch.append(lambda g=g, b=b: k_chain(g, b))
                for h in range(QH):
                    ch.append(lambda h=h, b=b: q_chain(h, b))
                for sub in range(2):
                    ch.append(lambda s=sub, b=b: v_chain(s, b))
                return ch

            # ---------------- attention (B stream) ----------------
            def build_job(h, r, qt):
                """Return list of emission closures for one (head, chunk) job."""
                g = h // 2
                nkt = 4 * r + 4
                state = {"pts": {}, "acc": None, "at": None}

                def att_mm(mk):
                    nc.tensor.matmul(
                        state["at"][:],
                        V[g][:, mk * 128:(mk + 1) * 128],
                        state["pts"][mk][:],
                        start=(mk == 0), stop=(mk == nkt - 1),
                        skip_group_check=True)

                def tile_unit(mk):
                    o = mk - 4 * r
                    c0 = max(0, o) * 128
                    w = 512 - c0
                    sc = sc_ps.tile([128, 512], F32, tag="sc", name="sc")
                    nc.tensor.matmul(sc[:, 0:w],
                                     KT[g][:, mk * 128:(mk + 1) * 128],
                                     qt[:, c0:512], start=True, stop=True)
                    tt = ttp.tile([128, 512], F32, tag="tt", name="tt")
                    nc.scalar.activation(tt[:, 0:w], sc[:, 0:w], AF.Tanh,
                                         scale=SCALE / SOFTCAP)
                    pt = ptp.tile([128, 512], BF16, tag="pt", name="pt")
                    state["pts"][mk] = pt
                    nc.scalar.activation(pt[:, c0:512], tt[:, 0:w], AF.Exp,
                                         scale=SOFTCAP)
                    if o >= 0:
                        if c0 > 0:
                            nc.vector.memset(pt[:, 0:c0], 0.0)
                        nc.vector.tensor_mul(pt[:, c0:512], pt[:, c0:512],
                                             mask0[:, 0:w])
                    if mk == 0:
                        state["at"] = at_ps.tile([128, 512], F32, tag="at", name="at")
                        state["acc"] = accp.tile([128, 512], F32, tag="acc", name="acc")
                        nc.vector.tensor_copy(state["acc"][:], pt[:])
                    else:
                        nc.vector.tensor_add(state["acc"][:], state["acc"][:], pt[:])
                    if mk - ATT_LAG >= 0:
                        att_mm(mk - ATT_LAG)

                def job_end():
                    for mk in range(max(0, nkt - ATT_LAG), nkt):
                        att_mm(mk)
                    dn = at_ps.tile([128, 512], F32, tag="at", name="dn")
                    nc.tensor.matmul(dn[0:1, 0:512], ones[:], state["acc"][:],
                                     start=True, stop=True)
                    rc = rsb.tile([1, 512], F32, tag="rc", name="rc")
                    nc.vector.reciprocal(rc[:], dn[0:1, 0:512])
                    rb = rbc.tile([128, 512], F32, tag="rb", name="rb")
                    nc.gpsimd.partition_broadcast(rb[:], rc[:])
                    at_sb = atsb.tile([128, 512], BF16, tag=f"at{h}", name=f"at{h}")
                    nc.vector.tensor_mul(at_sb[:], state["at"][:], rb[:])
                    at_store[h] = at_sb

                units = [lambda mk=mk: tile_unit(mk) for mk in range(nkt)]
                units.append(job_end)
                return units

            # ---------------- o_proj (C stream) ----------------
            def oproj_group(r_prev, s, j, at_prev):
                po = op_ps.tile([128, 512], F32, tag="op", name="po")
                for h in range(QH):
                    nc.tensor.matmul(
                        po[:], at_prev[h][:, s * 128:(s + 1) * 128],
                        WO[h][:, j * 512:(j + 1) * 512],
                        start=(h == 0), stop=(h == QH - 1))
                ob = ost.tile([128, 512], F32, tag="ob", name="ob")
                nc.vector.tensor_copy(ob[:], po[:])
                row = r_prev * 512 + s * 128
                nc.sync.dma_start(out_d[row:row + 128, j * 512:(j + 1) * 512], ob[:])

            # ---------------- weave driver ----------------
            at_store = {}

            # prologue: projections for q-chunk 0 (blocks 0, 1).  The x-block
            # DMA for block b reuses block b-2's SBUF slot (xcol bufs=2), so it
            # must be emitted after block b-2's last reader (WAR tracking is
            # emission-ordered).
            for ch in block_chains(0):
                ch()
            dma_x_block(2)
            for ch in block_chains(1):
                ch()
            dma_x_block(3)

            for r in range(NQ):
                # B stream: attention jobs for chunk r (qt APs captured now)
                b_units = []
                for h in range(QH):
                    b_units.extend(build_job(h, r, qtn_cur[h]))

                # fillers: A = proj chains for chunk r+1, C = o_proj chunk r-1.
                # The prefetch DMA for block b+2 is emitted right after block
                # b's chains (its slot's last emitted readers).
                a_items = []
                if r + 1 < NQ:
                    for b in (2 * r + 2, 2 * r + 3):
                        a_items.extend(block_chains(b))
                        if b + 2 < NB:
                            a_items.append(lambda b2=b + 2: dma_x_block(b2))
                c_items = []
                if r >= 1:
                    at_prev = dict(at_store)
                    for s in range(4):
                        for j in range(DOUT_CHUNKS):
                            c_items.append(
                                lambda r=r, s=s, j=j, ap=at_prev:
                                oproj_group(r - 1, s, j, ap))
                # merge A and C round-robin, ~2 C per A (PE-time balanced)
                fillers = []
                ai, ci = 0, 0
                while ai < len(a_items) or ci < len(c_items):
                    for _ in range(2):
                        if ci < len(c_items):
                            fillers.append(c_items[ci]); ci += 1
                    if ai < len(a_items):
                        fillers.append(a_items[ai]); ai += 1

                # weave: emit fillers between every 2 B units, spread evenly
                n_slots = max(1, len(b_units) // 2)
                fi = 0
                slot = 0
                for i, u in enumerate(b_units):
                    u()
                    if i % 2 == 1:
                        slot += 1
                        want = (len(fillers) * slot) // n_slots
                        while fi < want:
                            fillers[fi]()
                            fi += 1
                while fi < len(fillers):
                    fillers[fi]()
                    fi += 1

            # epilogue: o_proj for the last chunk
            at_prev = dict(at_store)
            for s in range(4):
                for j in range(DOUT_CHUNKS):
                    oproj_group(NQ - 1, s, j, at_prev)
    return nc


_CACHED_NC = {}


def build(n_iters=1):
    if n_iters not in _CACHED_NC:
        nc = bacc.Bacc("TRN2", target_bir_lowering=False, debug=False)
        _emit(nc)
        nc.compile()
        _CACHED_NC[n_iters] = nc
    return _CACHED_NC[n_iters]


def host_tables():
    inv_freq = 1.0 / (ROPE_THETA ** (np.arange(0, HEAD_DIM, 2, dtype=np.float32) / HEAD_DIM))
    ang = np.arange(L, dtype=np.float32)[:, None] * inv_freq[None, :]  # [L, 64]
    cos, sin = np.cos(ang), np.sin(ang)
    cosT = np.concatenate([cos.T, cos.T], axis=0).astype(BF16_NP)
    sinT = np.concatenate([-sin.T, sin.T], axis=0).astype(BF16_NP)
    return np.ascontiguousarray(cosT), np.ascontiguousarray(sinT)


def host_mask():
    k = np.arange(128)[:, None]
    q = np.arange(512)[None, :]
    return np.ascontiguousarray((q >= k).astype(BF16_NP))


def make_in_maps(x, wq, wk, wv, wo):
    cosT, sinT = host_tables()
    mask = host_mask()
    xt = np.ascontiguousarray(x.reshape(L, D).T).astype(BF16_NP)
    in_maps = []
    for c in range(N_CORES):
        qs = slice(c * QH * 128, (c + 1) * QH * 128)
        kvs = slice(c * KVH * 128, (c + 1) * KVH * 128)
        in_maps.append({
            "xt": xt,
            "wqt": np.ascontiguousarray(wq[qs].T.astype(BF16_NP)),
            "wkt": np.ascontiguousarray(wk[kvs].T.astype(BF16_NP)),
            "wvt": np.ascontiguousarray(wv[kvs].T.astype(BF16_NP)),
            "wot": np.ascontiguousarray(wo[:, qs].T.astype(BF16_NP)),
            "cost": cosT,
            "sint": sinT,
            "mask": mask,
        })
    return in_maps


def run(inputs, trace=False, trace_kwargs=None):
    from concourse.bass_utils import run_bass_kernel_spmd

    nc = build()
    x = np.asarray(inputs["x"], dtype=np.float32)
    in_maps = make_in_maps(
        x,
        np.asarray(inputs["wq"], dtype=np.float32),
        np.asarray(inputs["wk"], dtype=np.float32),
        np.asarray(inputs["wv"], dtype=np.float32),
        np.asarray(inputs["wo"], dtype=np.float32),
    )
    res = run_bass_kernel_spmd(
        nc, in_maps, core_ids=list(range(N_CORES)),
        trace=trace, **(trace_kwargs or {}))
    out = np.zeros((L, D), dtype=np.float32)
    for c in range(N_CORES):
        out += res.results[c]["out"]
    return out.reshape(x.shape), res


def kernel(**inputs) -> np.ndarray:
    out, _ = run(inputs, trace=False)
    return out


# revision 11
# speedup vs baseline: 1.0410x; 1.0410x over previous
"""Trainium2 Bass kernel for GQA attention (32 q heads / 16 kv heads, head_dim
128, L=2048, D=4608) with RoPE, tanh softcap 50, causal mask, o_proj.

Tensor-parallel over heads across 8 NeuronCores; core c owns q-heads 4c..4c+3
and kv-heads 2c..2c+1; host sums the 8 partial [L, D] outputs.

v2: single fused software-pipelined schedule ("weave").  The kernel runs 4
rounds (one per 512-wide q chunk).  In round r the PE instruction stream
interleaves three workloads so the scalar-engine softcap chain (tanh+exp,
~178us total) always hides under ready matmul work:
  B: attention for q-chunk r (scores -> tanh -> exp -> mask -> PV)
  A: Q/K/V projections + rope for q-chunk r+1 (16 chains of 36 MMs @256)
  C: o_proj for q-chunk r-1 (36 groups of 4 MMs @512)

Other changes vs v1:
  - PV computes attnT [d, q] directly (lhsT = V tile, rhs = PT tile, 512-wide
    accumulation in one PSUM bank) - no PE transposes, no 129-wide matmuls.
  - softmax denominator: DVE accumulates sum of exp tiles in f32, one
    ones-vector matmul per (head, chunk) reduces over partitions; the
    reciprocal is broadcast (gpsimd) and folded into the attnT PSUM drain.
  - rope applied by DVE reading the projection PSUM directly (no ACT copies).
  - x streamed in 256-wide column blocks (halves SBUF double-buffer cost).
"""

import numpy as np
import ml_dtypes

import concourse.bass as bass
import concourse.mybir as mybir
import concourse.tile as tile
from concourse import bacc

F32 = mybir.dt.float32
BF16 = mybir.dt.bfloat16
BF16_NP = ml_dtypes.bfloat16
AF = mybir.ActivationFunctionType

N_HEADS = 32
N_KV = 16
HEAD_DIM = 128
ROPE_THETA = 10000.0
SOFTCAP = 50.0
SCALE = 1.0 / 12.0  # 1/sqrt(144)
L = 2048
D = 4608
N_CORES = 8
QH = N_HEADS // N_CORES        # 4 local q heads
KVH = N_KV // N_CORES          # 2 local kv heads
KC = D // 128                  # 36 contraction chunks
NB = L // 256                  # 8 projection column blocks (256 wide)
NQ = L // 512                  # 4 attention q-chunks (512 wide)
LT = L // 128                  # 16 k-tiles of 128
DOUT_CHUNKS = D // 512         # 9 o_proj output chunks
ATT_LAG = 3                    # attnT MM trails the exp by this many tiles


def _emit(nc):
    xt_d = nc.dram_tensor("xt", [D, L], BF16, kind="ExternalInput")
    wqt_d = nc.dram_tensor("wqt", [D, QH * 128], BF16, kind="ExternalInput")
    wkt_d = nc.dram_tensor("wkt", [D, KVH * 128], BF16, kind="ExternalInput")
    wvt_d = nc.dram_tensor("wvt", [D, KVH * 128], BF16, kind="ExternalInput")
    wot_d = nc.dram_tensor("wot", [QH * 128, D], BF16, kind="ExternalInput")
    cost_d = nc.dram_tensor("cost", [128, L], BF16, kind="ExternalInput")
    sint_d = nc.dram_tensor("sint", [128, L], BF16, kind="ExternalInput")
    mask_d = nc.dram_tensor("mask", [128, 512], BF16, kind="ExternalInput")
    out_d = nc.dram_tensor("out", [L, D], F32, kind="ExternalOutput")

    from contextlib import ExitStack

    with tile.TileContext(nc) as tc:
        with ExitStack() as stack:
            pool = lambda *a, **kw: stack.enter_context(tc.tile_pool(*a, **kw))
            const = pool(name="const", bufs=1)
            persist = pool(name="persist", bufs=1)
            qtn = pool(name="qtn", bufs=2)
            xcol = pool(name="xcol", bufs=2)
            wts = pool(name="wts", bufs=1)
            wop = pool(name="wop", bufs=1)
            rtmp = pool(name="rtmp", bufs=1)
            ttp = pool(name="ttp", bufs=1)
            ptp = pool(name="ptp", bufs=4)
            accp = pool(name="accp", bufs=2)
            atsb = pool(name="atsb", bufs=2)
            rsb = pool(name="rsb", bufs=1)
            rbc = pool(name="rbc", bufs=1)
            ost = pool(name="ost", bufs=2)
            pj_ps = pool(name="pj_ps", bufs=2, space="PSUM")
            sc_ps = pool(name="sc_ps", bufs=2, space="PSUM")
            at_ps = pool(name="at_ps", bufs=2, space="PSUM")
            op_ps = pool(name="op_ps", bufs=2, space="PSUM")
            # ---------------- constants ----------------
            ones = const.tile([128, 1], F32)
            nc.vector.memset(ones[:], 1.0)
            warm = const.tile([128, 2], F32)
            # preload the exp/tanh activation table set during the prologue
            nc.scalar.activation(warm[:, 0:1], ones[:], AF.Tanh)
            nc.scalar.activation(warm[:, 1:2], warm[:, 0:1], AF.Exp)

            cost = const.tile([128, L], BF16)
            sint = const.tile([128, L], BF16)
            mask0 = const.tile([128, 512], BF16)

            # persistent per-head tensors
            KT = [persist.tile([128, L], BF16, tag=f"kt{g}", name=f"kt{g}")
                  for g in range(KVH)]
            V = [persist.tile([128, LT * 128], BF16, tag=f"v{g}", name=f"v{g}")
                 for g in range(KVH)]

            # ---------------- input DMAs (prologue) ----------------
            wk, wq, wv = [], [], []
            xc_store = {}  # block -> [36 chunk APs]

            def dma_x_block(b):
                aps = []
                for k in range(KC):
                    t = xcol.tile([128, 256], BF16, tag=f"x{k}", name=f"xc{k}")
                    nc.sync.dma_start(
                        t[:], xt_d[k * 128:(k + 1) * 128, b * 256:(b + 1) * 256])
                    aps.append(t)
                xc_store[b] = aps

            # first K chain needs wk[k] + block0; interleave those DMAs
            for k in range(KC):
                w = wts.tile([128, KVH * 128], BF16, tag=f"k{k}", name=f"wk{k}")
                nc.sync.dma_start(w[:], wkt_d[k * 128:(k + 1) * 128, :])
                wk.append(w)
            dma_x_block(0)
            nc.sync.dma_start(cost[:], cost_d[:])
            nc.sync.dma_start(sint[:], sint_d[:])
            for k in range(KC):
                w = wts.tile([128, QH * 128], BF16, tag=f"q{k}", name=f"wq{k}")
                nc.sync.dma_start(w[:], wqt_d[k * 128:(k + 1) * 128, :])
                wq.append(w)
            dma_x_block(1)
            for k in range(KC):
                w = wts.tile([128, KVH * 128], BF16, tag=f"v{k}", name=f"wv{k}")
                nc.sync.dma_start(w[:], wvt_d[k * 128:(k + 1) * 128, :])
                wv.append(w)
            nc.sync.dma_start(mask0[:], mask_d[:])
            WO = []
            for h in range(QH):
                w = wop.tile([128, D], BF16, tag=f"wo{h}")
                nc.sync.dma_start(w[:], wot_d[h * 128:(h + 1) * 128, :])
                WO.append(w)

            # ---------------- projection chains ----------------
            qtn_cur = {}   # h -> current-round QT tile [128, 512]

            def rope_drain(ps, dst, b):
                cols = slice(b * 256, (b + 1) * 256)
                m1 = rtmp.tile([128, 256], F32, tag="m1", name="m1")
                nc.vector.tensor_mul(m1[0:64, :], ps[64:128, 0:256], sint[0:64, cols])
                nc.vector.tensor_mul(m1[64:128, :], ps[0:64, 0:256], sint[64:128, cols])
                m2 = rtmp.tile([128, 256], F32, tag="m2", name="m2")
                nc.vector.tensor_mul(m2[:], ps[:, 0:256], cost[:, cols])
                nc.vector.tensor_add(dst, m2[:], m1[:])

            def k_chain(g, b):
                xc = xc_store[b]
                ps = pj_ps.tile([128, 512], F32, tag="pj", name="ps")
                for k in range(KC):
                    nc.tensor.matmul(
                        ps[:, 0:256], wk[k][:, g * 128:(g + 1) * 128], xc[k][:],
                        start=(k == 0), stop=(k == KC - 1))
                rope_drain(ps, KT[g][:, b * 256:(b + 1) * 256], b)

            def q_chain(h, b):
                xc = xc_store[b]
                if b % 2 == 0:
                    qtn_cur[h] = qtn.tile([128, 512], BF16, tag=f"q{h}", name=f"qt{h}")
                ps = pj_ps.tile([128, 512], F32, tag="pj", name="ps")
                for k in range(KC):
                    nc.tensor.matmul(
                        ps[:, 0:256], wq[k][:, h * 128:(h + 1) * 128], xc[k][:],
                        start=(k == 0), stop=(k == KC - 1))
                c0 = (b % 2) * 256
                rope_drain(ps, qtn_cur[h][:, c0:c0 + 256], b)

            def v_chain(sub, b):
                xc = xc_store[b]
                ps = pj_ps.tile([128, 512], F32, tag="pj", name="ps")
                for k in range(KC):
                    nc.tensor.matmul(
                        ps[:, 0:256], xc[k][:, sub * 128:(sub + 1) * 128], wv[k][:],
                        start=(k == 0), stop=(k == KC - 1))
                mk = 2 * b + sub
                for g in range(KVH):
                    nc.vector.tensor_copy(
                        V[g][:, mk * 128:(mk + 1) * 128],
                        ps[:, g * 128:(g + 1) * 128])

            def block_chains(b):
                ch = []
                for g in range(KVH):
                    ch.append(lambda g=g, b=b: k_chain(g, b))
                for h in range(QH):
                    ch.append(lambda h=h, b=b: q_chain(h, b))
                for sub in range(2):
                    ch.append(lambda s=sub, b=b: v_chain(s, b))
                return ch

            # ---------------- attention (B stream) ----------------
            def build_job(h, r, qt):
                """Return list of emission closures for one (head, chunk) job."""
                g = h // 2
                nkt = 4 * r + 4
                state = {"pts": {}, "acc": None, "at": None}

                def att_mm(mk):
                    nc.tensor.matmul(
                        state["at"][:],
                        V[g][:, mk * 128:(mk + 1) * 128],
                        state["pts"][mk][:],
                        start=(mk == 0), stop=(mk == nkt - 1),
                        skip_group_check=True)

                def tile_unit(mk):
                    o = mk - 4 * r
                    c0 = max(0, o) * 128
                    w = 512 - c0
                    sc = sc_ps.tile([128, 512], F32, tag="sc", name="sc")
                    nc.tensor.matmul(sc[:, 0:w],
                                     KT[g][:, mk * 128:(mk + 1) * 128],
                                     qt[:, c0:512], start=True, stop=True)
                    tt = ttp.tile([128, 512], F32, tag="tt", name="tt")
                    nc.scalar.activation(tt[:, 0:w], sc[:, 0:w], AF.Tanh,
                                         scale=SCALE / SOFTCAP)
                    pt = ptp.tile([128, 512], BF16, tag="pt", name="pt")
                    state["pts"][mk] = pt
                    nc.scalar.activation(pt[:, c0:512], tt[:, 0:w], AF.Exp,
                                         scale=SOFTCAP)
                    if o >= 0:
                        if c0 > 0:
                            nc.vector.memset(pt[:, 0:c0], 0.0)
                        nc.vector.tensor_mul(pt[:, c0:512], pt[:, c0:512],
                                             mask0[:, 0:w])
                    if mk == 0:
                        state["at"] = at_ps.tile([128, 512], F32, tag="at", name="at")
                        state["acc"] = accp.tile([128, 512], F32, tag="acc", name="acc")
                        nc.gpsimd.tensor_copy(state["acc"][:], pt[:])
                    else:
                        nc.gpsimd.tensor_add(state["acc"][:], state["acc"][:], pt[:])
                    if mk - ATT_LAG >= 0:
                        att_mm(mk - ATT_LAG)

                def job_end():
                    for mk in range(max(0, nkt - ATT_LAG), nkt):
                        att_mm(mk)
                    dn = at_ps.tile([128, 512], F32, tag="at", name="dn")
                    nc.tensor.matmul(dn[0:1, 0:512], ones[:], state["acc"][:],
                                     start=True, stop=True)
                    rc = rsb.tile([1, 512], F32, tag="rc", name="rc")
                    nc.vector.reciprocal_approx_fast(rc[:], dn[0:1, 0:512])
                    rb = rbc.tile([128, 512], F32, tag="rb", name="rb")
                    nc.gpsimd.partition_broadcast(rb[:], rc[:])
                    at_sb = atsb.tile([128, 512], BF16, tag=f"at{h}", name=f"at{h}")
                    nc.vector.tensor_mul(at_sb[:], state["at"][:], rb[:])
                    at_store[h] = at_sb

                units = [lambda mk=mk: tile_unit(mk) for mk in range(nkt)]
                units.append(job_end)
                return units

            # ---------------- o_proj (C stream) ----------------
            def oproj_group(r_prev, s, j, at_prev):
                po = op_ps.tile([128, 512], F32, tag="op", name="po")
                for h in range(QH):
                    nc.tensor.matmul(
                        po[:], at_prev[h][:, s * 128:(s + 1) * 128],
                        WO[h][:, j * 512:(j + 1) * 512],
                        start=(h == 0), stop=(h == QH - 1))
                ob = ost.tile([128, 512], F32, tag="ob", name="ob")
                nc.vector.tensor_copy(ob[:], po[:])
                row = r_prev * 512 + s * 128
                nc.sync.dma_start(out_d[row:row + 128, j * 512:(j + 1) * 512], ob[:])

            # ---------------- weave driver ----------------
            at_store = {}

            # prologue: projections for q-chunk 0 (blocks 0, 1).  The x-block
            # DMA for block b reuses block b-2's SBUF slot (xcol bufs=2), so it
            # must be emitted after block b-2's last reader (WAR tracking is
            # emission-ordered).
            for ch in block_chains(0):
                ch()
            dma_x_block(2)
            for ch in block_chains(1):
                ch()
            dma_x_block(3)

            for r in range(NQ):
                # B stream: attention jobs for chunk r (qt APs captured now)
                b_units = []
                for h in range(QH):
                    b_units.extend(build_job(h, r, qtn_cur[h]))

                # fillers: A = proj chains for chunk r+1, C = o_proj chunk r-1.
                # The prefetch DMA for block b+2 is emitted right after block
                # b's chains (its slot's last emitted readers).
                a_items = []
                if r + 1 < NQ:
                    for b in (2 * r + 2, 2 * r + 3):
                        a_items.extend(block_chains(b))
                        if b + 2 < NB:
                            a_items.append(lambda b2=b + 2: dma_x_block(b2))
                c_items = []
                if r >= 1:
                    at_prev = dict(at_store)
                    for s in range(4):
                        for j in range(DOUT_CHUNKS):
                            c_items.append(
                                lambda r=r, s=s, j=j, ap=at_prev:
                                oproj_group(r - 1, s, j, ap))
                # merge A and C round-robin, ~2 C per A (PE-time balanced)
                fillers = []
                ai, ci = 0, 0
                while ai < len(a_items) or ci < len(c_items):
                    for _ in range(2):
                        if ci < len(c_items):
                            fillers.append(c_items[ci]); ci += 1
                    if ai < len(a_items):
                        fillers.append(a_items[ai]); ai += 1

                # weave: emit fillers between every 2 B units, spread evenly
                n_slots = max(1, len(b_units) // 2)
                fi = 0
                slot = 0
                for i, u in enumerate(b_units):
                    u()
                    if i % 2 == 1:
                        slot += 1
                        want = (len(fillers) * slot) // n_slots
                        while fi < want:
                            fillers[fi]()
                            fi += 1
                while fi < len(fillers):
                    fillers[fi]()
                    fi += 1

            # epilogue: o_proj for the last chunk
            at_prev = dict(at_store)
            for s in range(4):
                for j in range(DOUT_CHUNKS):
                    oproj_group(NQ - 1, s, j, at_prev)
    return nc


_CACHED_NC = {}


def build(n_iters=1):
    if n_iters not in _CACHED_NC:
        nc = bacc.Bacc("TRN2", target_bir_lowering=False, debug=False)
        _emit(nc)
        nc.compile()
        _CACHED_NC[n_iters] = nc
    return _CACHED_NC[n_iters]


def host_tables():
    inv_freq = 1.0 / (ROPE_THETA ** (np.arange(0, HEAD_DIM, 2, dtype=np.float32) / HEAD_DIM))
    ang = np.arange(L, dtype=np.float32)[:, None] * inv_freq[None, :]  # [L, 64]
    cos, sin = np.cos(ang), np.sin(ang)
    cosT = np.concatenate([cos.T, cos.T], axis=0).astype(BF16_NP)
    sinT = np.concatenate([-sin.T, sin.T], axis=0).astype(BF16_NP)
    return np.ascontiguousarray(cosT), np.ascontiguousarray(sinT)


def host_mask():
    k = np.arange(128)[:, None]
    q = np.arange(512)[None, :]
    return np.ascontiguousarray((q >= k).astype(BF16_NP))


def make_in_maps(x, wq, wk, wv, wo):
    cosT, sinT = host_tables()
    mask = host_mask()
    xt = np.ascontiguousarray(x.reshape(L, D).T).astype(BF16_NP)
    in_maps = []
    for c in range(N_CORES):
        qs = slice(c * QH * 128, (c + 1) * QH * 128)
        kvs = slice(c * KVH * 128, (c + 1) * KVH * 128)
        in_maps.append({
            "xt": xt,
            "wqt": np.ascontiguousarray(wq[qs].T.astype(BF16_NP)),
            "wkt": np.ascontiguousarray(wk[kvs].T.astype(BF16_NP)),
            "wvt": np.ascontiguousarray(wv[kvs].T.astype(BF16_NP)),
            "wot": np.ascontiguousarray(wo[:, qs].T.astype(BF16_NP)),
            "cost": cosT,
            "sint": sinT,
            "mask": mask,
        })
    return in_maps


def run(inputs, trace=False, trace_kwargs=None):
    from concourse.bass_utils import run_bass_kernel_spmd

    nc = build()
    x = np.asarray(inputs["x"], dtype=np.float32)
    in_maps = make_in_maps(
        x,
        np.asarray(inputs["wq"], dtype=np.float32),
        np.asarray(inputs["wk"], dtype=np.float32),
        np.asarray(inputs["wv"], dtype=np.float32),
        np.asarray(inputs["wo"], dtype=np.float32),
    )
    res = run_bass_kernel_spmd(
        nc, in_maps, core_ids=list(range(N_CORES)),
        trace=trace, **(trace_kwargs or {}))
    out = np.zeros((L, D), dtype=np.float32)
    for c in range(N_CORES):
        out += res.results[c]["out"]
    return out.reshape(x.shape), res


def kernel(**inputs) -> np.ndarray:
    out, _ = run(inputs, trace=False)
    return out
